# revision 1
# baseline (speedup 1.0000x reference)
"""FBPINN (16 subdomain MLPs over [0,1]^2, cosine partition-of-unity windows)
as a Trainium2 Bass kernel, expert-parallel over 8 NeuronCores.

Sharding: subdomain axis K=16 split 2-per-core. Each core runs its two
subnets on all N points and emits partial (numerator, denominator) sums of
the window-weighted combine; the host adds partials across cores and divides.
This is exact because the window normalization is a ratio of K-sums.

Host-side preprocessing folds the per-subdomain input normalization
xn = (x - center)/scale into the layer-0 weights/bias, so the device
only computes tanh-MLP layers + windows. Matmuls run in float32r
(single-pass PE mode, ~4x the fp32 rate; ~1e-3 relative rounding).
"""

import contextlib
import math

import numpy as np

import concourse.bacc as bacc
import concourse.mybir as mybir
import concourse.tile as tile
from concourse.bass_utils import run_bass_kernel_spmd

# problem constants (hardcoded per harness contract)
K, D, N, W, OUT_DIM = 16, 2, 16384, 256, 1
TW = 0.2
NCORES = 8
KPC = K // NCORES  # subdomains per core
P = 128
NT = N // P        # 128 columns in point-partition layout
CH = 1024          # point chunk through the MLP pipeline
NCH = N // CH
SUB = 512          # matmul moving-operand subchunk
FT = W // P        # feature tiles per hidden layer (2)

F32 = mybir.dt.float32
F32R = mybir.dt.float32r
AF = mybir.ActivationFunctionType
ALU = mybir.AluOpType


def _build_program(repeat=1):
    nc = bacc.Bacc("TRN2", target_bir_lowering=False, debug=False, num_devices=NCORES)

    xT = nc.dram_tensor("XT", [D, N], F32R, kind="ExternalInput")
    xPT = nc.dram_tensor("XPT", [P, D * NT], F32, kind="ExternalInput")
    w0d = nc.dram_tensor("W0S", [D, KPC * W], F32R, kind="ExternalInput")
    b0d = nc.dram_tensor("B0S", [P, KPC * FT], F32, kind="ExternalInput")
    w1d = nc.dram_tensor("W1S", [P, KPC * FT * FT, P], F32R, kind="ExternalInput")
    b1d = nc.dram_tensor("B1S", [P, KPC * FT], F32, kind="ExternalInput")
    w2d = nc.dram_tensor("W2S", [P, KPC * FT * FT, P], F32R, kind="ExternalInput")
    b2d = nc.dram_tensor("B2S", [P, KPC * FT], F32, kind="ExternalInput")
    w3d = nc.dram_tensor("W3S", [P, KPC * FT], F32, kind="ExternalInput")
    b3d = nc.dram_tensor("B3S", [P, KPC], F32, kind="ExternalInput")  # b3/128
    wbd = nc.dram_tensor("WBS", [P, KPC * 2 * D], F32, kind="ExternalInput")
    outd = nc.dram_tensor("OUT", [2, P, NT], F32, kind="ExternalOutput")

    scratch = nc.dram_tensor("SUBSCRATCH", [KPC, 1, N], F32)

    with tile.TileContext(nc) as tc:
        with (
            tc.tile_pool(name="const", bufs=1) as const,
            tc.tile_pool(name="win", bufs=1) as win,
            tc.tile_pool(name="wtmp", bufs=2) as wtmp,
            tc.tile_pool(name="xin", bufs=3) as xin,
            tc.tile_pool(name="hbuf", bufs=2) as hbuf,
            tc.tile_pool(name="sfold", bufs=2) as spool,
            tc.tile_pool(name="stage", bufs=3) as stage,
            tc.tile_pool(name="fin", bufs=1) as fin,
            tc.tile_pool(name="psum", bufs=3, space="PSUM") as psum,
            tc.tile_pool(name="psum_s", bufs=2, space="PSUM") as psum_s,
        ):
            # resident constants: small tensors first on the sync queue so
            # layer-0 can start immediately; the two big hidden-layer weight
            # loads go on the gpsimd DMA queue to stream in parallel.
            w0 = const.tile([D, KPC * W], F32R)
            nc.sync.dma_start(w0[:], w0d[:])
            b0 = const.tile([P, KPC * FT], F32)
            nc.sync.dma_start(b0[:], b0d[:])
            b1 = const.tile([P, KPC * FT], F32)
            nc.sync.dma_start(b1[:], b1d[:])
            b2 = const.tile([P, KPC * FT], F32)
            nc.sync.dma_start(b2[:], b2d[:])
            w3 = const.tile([P, KPC * FT], F32)
            nc.sync.dma_start(w3[:], w3d[:])
            b3 = const.tile([P, KPC], F32)
            nc.sync.dma_start(b3[:], b3d[:])
            wb = const.tile([P, KPC * 2 * D], F32)
            nc.sync.dma_start(wb[:], wbd[:])
            xpt = const.tile([P, D * NT], F32)
            nc.sync.dma_start(xpt[:], xPT[:])
            w1 = const.tile([P, KPC * FT * FT, P], F32R)
            nc.gpsimd.dma_start(w1[:], w1d[:])
            w2 = const.tile([P, KPC * FT * FT, P], F32R)
            nc.gpsimd.dma_start(w2[:], w2d[:])
            ones_f = const.tile([P, 1], F32)
            nc.vector.memset(ones_f[:], 1.0)
            ones = const.tile([P, 1], F32R)
            nc.vector.tensor_copy(ones[:], ones_f[:])

            loop = tc.For_i(0, repeat, 1) if repeat > 1 else contextlib.nullcontext()
            with loop:
                # main MLP pipeline, chunked over points; the two
                # subdomain streams are interleaved stage-by-stage so the PE
                # works on stream B's matmuls while ACT drains stream A.
                for c in range(NCH):
                    x2c = xin.tile([D, CH], F32R)
                    nc.sync.dma_start(x2c[:], xT[:, c * CH:(c + 1) * CH])
                    # layer 0, both streams
                    ps0 = {}
                    for s in range(KPC):
                        for mt in range(FT):
                            pt = psum.tile([P, CH], F32, tag="mm")
                            for j in range(CH // SUB):
                                js = slice(j * SUB, (j + 1) * SUB)
                                nc.tensor.matmul(
                                    pt[:, js],
                                    w0[:, (s * FT + mt) * P:(s * FT + mt + 1) * P],
                                    x2c[:, js],
                                    start=True, stop=True,
                                )
                            ps0[s, mt] = pt
                    hcur = {}
                    for s in range(KPC):
                        h0 = hbuf.tile([P, FT, CH], F32R, tag=f"h0_{s}")
                        for mt in range(FT):
                            nc.scalar.activation(
                                h0[:, mt, :], ps0[s, mt][:], AF.Tanh,
                                bias=b0[:, s * FT + mt:s * FT + mt + 1],
                            )
                        hcur[s] = h0
                    # hidden layers 1 and 2, both streams per stage
                    for wl, bl, htag in ((w1, b1, "h1"), (w2, b2, "h2")):
                        psl = {}
                        for s in range(KPC):
                            for mt in range(FT):
                                pt = psum.tile([P, CH], F32, tag="mm")
                                for j in range(CH // SUB):
                                    js = slice(j * SUB, (j + 1) * SUB)
                                    for ct in range(FT):
                                        nc.tensor.matmul(
                                            pt[:, js],
                                            wl[:, (s * FT + mt) * FT + ct, :],
                                            hcur[s][:, ct, js],
                                            start=(ct == 0), stop=(ct == FT - 1),
                                        )
                                psl[s, mt] = pt
                        hnxt = {}
                        for s in range(KPC):
                            hn = hbuf.tile([P, FT, CH], F32R, tag=f"{htag}_{s}")
                            for mt in range(FT):
                                nc.scalar.activation(
                                    hn[:, mt, :], psl[s, mt][:], AF.Tanh,
                                    bias=bl[:, s * FT + mt:s * FT + mt + 1],
                                )
                            hnxt[s] = hn
                        hcur = hnxt
                    # W3 fold + partition reduction via ones-matmul
                    for s in range(KPC):
                        sf = spool.tile([P, CH], F32R, tag=f"sf_{s}")
                        nc.vector.tensor_scalar(
                            sf[:], hcur[s][:, 0, :],
                            w3[:, s * FT:s * FT + 1], b3[:, s:s + 1],
                            op0=ALU.mult, op1=ALU.add,
                        )
                        sf2 = spool.tile([P, CH], F32R, tag=f"sf2_{s}")
                        nc.vector.tensor_scalar(
                            sf2[:], hcur[s][:, 1, :],
                            w3[:, s * FT + 1:s * FT + 2], None, op0=ALU.mult,
                        )
                        nc.vector.tensor_tensor(sf[:], sf[:], sf2[:], ALU.add)
                        for j in range(CH // SUB):
                            js = slice(j * SUB, (j + 1) * SUB)
                            pss = psum_s.tile([1, SUB], F32, tag="ps_s")
                            nc.tensor.matmul(
                                pss[:], ones[:], sf[:, js], start=True, stop=True,
                            )
                            row = stage.tile([1, SUB], F32, tag="row")
                            nc.vector.tensor_copy(row[:], pss[:])
                            off = c * CH + j * SUB
                            nc.sync.dma_start(
                                scratch[s, :, off:off + SUB], row[:],
                            )

                # cosine partition-of-unity windows, point-partition layout.
                # 0.25*(1-cos(pi a))*(1-cos(pi b)) = (sin(pi a/2)*sin(pi b/2))^2
                # so w_raw = (prod over 4 ramp sides of sin(pi t/2))^2, exactly.
                wvals = win.tile([P, KPC, NT], F32)
                for s in range(KPC):
                    vs = []
                    for d in range(D):
                        for side in range(2):  # 0 = left ramp, 1 = right ramp
                            col = s * 4 + side * 2 + d
                            u = wtmp.tile([P, NT], F32, tag="w_u")
                            sc = 2.5 if side == 0 else -2.5  # 1/(2*TW)
                            nc.vector.tensor_scalar(
                                u[:], xpt[:, d * NT:(d + 1) * NT],
                                sc, wb[:, col:col + 1], op0=ALU.mult, op1=ALU.add,
                            )
                            nc.vector.tensor_scalar(
                                u[:], u[:], 1.0, 0.0, op0=ALU.min, op1=ALU.max,
                            )
                            v = wtmp.tile([P, NT], F32, tag=f"w_v{side}{d}")
                            nc.scalar.activation(
                                v[:], u[:], AF.Sin, scale=math.pi / 2,
                            )
                            vs.append(v)
                    pa = wtmp.tile([P, NT], F32, tag="w_pa")
                    nc.vector.tensor_tensor(pa[:], vs[0][:], vs[1][:], ALU.mult)
                    pb = wtmp.tile([P, NT], F32, tag="w_pb")
                    nc.vector.tensor_tensor(pb[:], vs[2][:], vs[3][:], ALU.mult)
                    nc.vector.tensor_tensor(pa[:], pa[:], pb[:], ALU.mult)
                    nc.vector.tensor_tensor(wvals[:, s, :], pa[:], pa[:], ALU.mult)

                # final combine: num = sum_s sub_s * w_s ; den = sum_s w_s
                num = fin.tile([P, NT], F32)
                den = fin.tile([P, NT], F32)
                for s in range(KPC):
                    subf = fin.tile([P, NT], F32, tag=f"subf{s}")
                    nc.sync.dma_start(
                        subf[:], scratch[s, 0, :].rearrange("(p t) -> p t", p=P),
                    )
                    prod = fin.tile([P, NT], F32, tag=f"prod{s}")
                    nc.vector.tensor_tensor(
                        prod[:], subf[:], wvals[:, s, :], ALU.mult,
                    )
                    if s == 0:
                        nc.vector.tensor_copy(num[:], prod[:])
                        nc.vector.tensor_copy(den[:], wvals[:, s, :])
                    else:
                        nc.vector.tensor_tensor(num[:], num[:], prod[:], ALU.add)
                        nc.vector.tensor_tensor(
                            den[:], den[:], wvals[:, s, :], ALU.add,
                        )
                nc.sync.dma_start(outd[0], num[:])
                nc.sync.dma_start(outd[1], den[:])

    nc.compile()
    return nc


_PROGRAM = None


def _program():
    global _PROGRAM
    if _PROGRAM is None:
        _PROGRAM = _build_program()
    return _PROGRAM


def _prep_in_maps(x, W0, b0, W1, b1, W2, b2, W3, b3, xmins, xmaxs):
    f32 = np.float32
    x = np.asarray(x, f32)
    center = (xmins + xmaxs) * 0.5
    scale = np.maximum((xmaxs - xmins) * 0.5, 1e-9).astype(f32)

    xT = np.ascontiguousarray(x.T)  # [D, N]
    # point-partition layout: xpt[p, d*NT + t] = x[p*NT + t, d]
    # p-major point mapping: n = p*NT + t  ->  xpt[p, d*NT + t] = x[p*NT + t, d]
    xpt = np.ascontiguousarray(
        x.reshape(P, NT, D).transpose(0, 2, 1).reshape(P, D * NT)
    )

    in_maps = []
    for core in range(NCORES):
        ks = [core * KPC + s for s in range(KPC)]
        w0s = np.empty((D, KPC * W), f32)
        b0s = np.empty((P, KPC * FT), f32)
        w1s = np.empty((P, KPC * FT * FT, P), f32)
        b1s = np.empty((P, KPC * FT), f32)
        w2s = np.empty((P, KPC * FT * FT, P), f32)
        b2s = np.empty((P, KPC * FT), f32)
        w3s = np.empty((P, KPC * FT), f32)
        b3s = np.empty((P, KPC), f32)
        wbs = np.empty((P, KPC * 2 * D), f32)
        for s, k in enumerate(ks):
            # fold input normalization into layer 0
            w0eff = (W0[k] / scale[k][:, None]).astype(f32)  # [D, W]
            b0eff = (b0[k] - (center[k] / scale[k]) @ W0[k]).astype(f32)  # [W]
            w0s[:, s * W:(s + 1) * W] = w0eff
            for mt in range(FT):
                b0s[:, s * FT + mt] = b0eff[mt * P:(mt + 1) * P]
                b1s[:, s * FT + mt] = b1[k][mt * P:(mt + 1) * P]
                b2s[:, s * FT + mt] = b2[k][mt * P:(mt + 1) * P]
                w3s[:, s * FT + mt] = W3[k][mt * P:(mt + 1) * P, 0]
                for ct in range(FT):
                    w1s[:, (s * FT + mt) * FT + ct, :] = (
                        W1[k][ct * P:(ct + 1) * P, mt * P:(mt + 1) * P]
                    )
                    w2s[:, (s * FT + mt) * FT + ct, :] = (
                        W2[k][ct * P:(ct + 1) * P, mt * P:(mt + 1) * P]
                    )
            b3s[:, s] = b3[k][0] / P
            for dd in range(D):
                # left: u = x*2.5 + (TW - xmin)/(2 TW)
                wbs[:, s * 4 + 0 * 2 + dd] = (TW - xmins[k, dd]) / (2 * TW)
                # right: u = -x*2.5 + (xmax + TW)/(2 TW)
                wbs[:, s * 4 + 1 * 2 + dd] = (xmaxs[k, dd] + TW) / (2 * TW)
        in_maps.append({
            "XT": xT, "XPT": xpt,
            "W0S": w0s, "B0S": b0s,
            "W1S": w1s, "B1S": b1s,
            "W2S": w2s, "B2S": b2s,
            "W3S": w3s, "B3S": b3s,
            "WBS": wbs,
        })
    return in_maps


def kernel(x, W0, b0, W1, b1, W2, b2, W3, b3, xmins, xmaxs):
    args = [np.asarray(a, np.float32) for a in
            (x, W0, b0, W1, b1, W2, b2, W3, b3, xmins, xmaxs)]
    in_maps = _prep_in_maps(*args)
    nc = _program()
    res = run_bass_kernel_spmd(nc, in_maps, list(range(NCORES)))
    num = np.zeros((P, NT), np.float64)
    den = np.zeros((P, NT), np.float64)
    for i in range(NCORES):
        out = res.results[i]["OUT"]
        num += out[0]
        den += out[1]
    num = num.astype(np.float32)
    den = den.astype(np.float32)
    result = num / (den + np.float32(1e-9))
    # invert p-major mapping: n = p*NT + t
    return result.reshape(N, OUT_DIM).astype(np.float32)



# revision 3
# speedup vs baseline: 1.3422x; 1.3422x over previous
"""FBPINN (16 subdomain MLPs over [0,1]^2, cosine partition-of-unity windows)
as a Trainium2 Bass kernel with MoE-style routing across 8 NeuronCores.

Key idea: each subdomain's window is exactly zero outside its support box
(xmin-TW, xmax+TW), so its MLP only needs to run on the ~20-42% of points
inside that box. The host routes: it gathers each subdomain's active points
into 1024-point blocks (~84 blocks total vs 256 dense), load-balances the
blocks across the 8 cores, and does the final window-weighted scatter/
normalize. The device runs the heavy part: per block, a 3-layer tanh MLP
(256 wide) plus the W3 contraction, with the per-subdomain input
normalization and layer-0/1/2 biases folded into matmuls so each layer is
a single fused tanh activation over both 128-feature tiles.

Per-core HW work drops ~3.3x vs dense; both TensorE and ScalarE stay busy.
"""

import numpy as np

import concourse.bacc as bacc
import concourse.mybir as mybir
import concourse.tile as tile
from concourse.bass_utils import run_bass_kernel_spmd

K, D, N, W, OUT_DIM = 16, 2, 16384, 256, 1
TW = 0.2
NCORES = 8
P = 128
CH = 1024          # points per block
HALF = 512         # matmul moving-operand subchunk (one PSUM bank)
FT = W // P        # feature tiles per hidden layer (2)

F32 = mybir.dt.float32
F32R = mybir.dt.float32r
AF = mybir.ActivationFunctionType
ALU = mybir.AluOpType


def _build_program(nblk):
    nc = bacc.Bacc("TRN2", target_bir_lowering=False, debug=False,
                   num_devices=NCORES)

    xad = nc.dram_tensor("XA", [3, nblk * CH], F32R, kind="ExternalInput")
    w0d = nc.dram_tensor("W0S", [35, nblk * P], F32R, kind="ExternalInput")
    w1d = nc.dram_tensor("W1S", [P, nblk * FT * FT, P], F32R, kind="ExternalInput")
    b1d = nc.dram_tensor("B1S", [33, nblk * P], F32R, kind="ExternalInput")
    w2d = nc.dram_tensor("W2S", [P, nblk * FT * FT, P], F32R, kind="ExternalInput")
    b2d = nc.dram_tensor("B2S", [33, nblk * P], F32R, kind="ExternalInput")
    w3d = nc.dram_tensor("W3S", [P, nblk * FT], F32R, kind="ExternalInput")
    outd = nc.dram_tensor("OUT", [nblk, 1, CH], F32, kind="ExternalOutput")

    with tile.TileContext(nc) as tc:
        with (
            tc.tile_pool(name="const", bufs=1) as const,
            tc.tile_pool(name="xin", bufs=nblk) as xin,
            tc.tile_pool(name="wgt", bufs=nblk) as wgt,
            tc.tile_pool(name="hbuf", bufs=4) as hbuf,
            tc.tile_pool(name="stage", bufs=4) as stage,
            tc.tile_pool(name="psum", bufs=2, space="PSUM") as psum,
        ):
            ones_f = const.tile([33, HALF], F32)
            nc.vector.memset(ones_f[:], 1.0)
            ones = const.tile([33, HALF], F32R)
            nc.vector.tensor_copy(ones[:], ones_f[:])

            # per-block inputs, all DMAs issued up front (block-major so
            # early blocks unblock quickly); big hidden weights on the
            # gpsimd queue, the rest on sync.
            xa, w0, w1, b1, w2, b2, w3 = [], [], [], [], [], [], []
            for b in range(nblk):
                xt = xin.tile([35, CH], F32R, tag="xa")
                nc.sync.dma_start(xt[0:3, :], xad[:, b * CH:(b + 1) * CH])
                nc.sync.dma_start(xt[32:35, :], xt[0:3, :])
                xa.append(xt)
                w0t = wgt.tile([35, P], F32R, tag="w0")
                nc.sync.dma_start(w0t[:], w0d[:, b * P:(b + 1) * P])
                w0.append(w0t)
                b1t = wgt.tile([33, P], F32R, tag="b1")
                nc.sync.dma_start(b1t[:], b1d[:, b * P:(b + 1) * P])
                b1.append(b1t)
                b2t = wgt.tile([33, P], F32R, tag="b2")
                nc.sync.dma_start(b2t[:], b2d[:, b * P:(b + 1) * P])
                b2.append(b2t)
                w3t = wgt.tile([P, FT], F32R, tag="w3")
                nc.sync.dma_start(w3t[:], w3d[:, b * FT:(b + 1) * FT])
                w3.append(w3t)
                w1t = wgt.tile([P, FT * FT, P], F32R, tag="w1")
                nc.gpsimd.dma_start(w1t[:], w1d[:, b * FT * FT:(b + 1) * FT * FT, :])
                w1.append(w1t)
                w2t = wgt.tile([P, FT * FT, P], F32R, tag="w2")
                nc.gpsimd.dma_start(w2t[:], w2d[:, b * FT * FT:(b + 1) * FT * FT, :])
                w2.append(w2t)

            def l0_mms(b):
                # layer 0: K=3 (two normalized coords + ones row carrying
                # b0); mt pair packed into PE row groups 0 / 32.
                pt = psum.tile([P, FT, CH], F32, tag="mm")
                for j in range(CH // HALF):
                    js = slice(j * HALF, (j + 1) * HALF)
                    nc.tensor.matmul(pt[:, 0, js], w0[b][0:3, :], xa[b][0:3, js],
                                     start=True, stop=True, tile_position=(0, 0))
                    nc.tensor.matmul(pt[:, 1, js], w0[b][32:35, :], xa[b][32:35, js],
                                     start=True, stop=True, tile_position=(32, 0))
                return pt

            def hidden_mms(b, wl, bl, h):
                # bias rows via K=1 matmuls (packed rows 0/32), then the
                # two K=128 contraction tiles accumulate on top.
                pt = psum.tile([P, FT, CH], F32, tag="mm")
                for j in range(CH // HALF):
                    js = slice(j * HALF, (j + 1) * HALF)
                    nc.tensor.matmul(pt[:, 0, js], bl[b][0:1, :], ones[0:1, :],
                                     start=True, stop=False, tile_position=(0, 0))
                    nc.tensor.matmul(pt[:, 1, js], bl[b][32:33, :], ones[32:33, :],
                                     start=True, stop=False, tile_position=(32, 0))
                    for mt in range(FT):
                        for ct in range(FT):
                            nc.tensor.matmul(
                                pt[:, mt, js], wl[b][:, mt * FT + ct, :],
                                h[:, ct, js],
                                start=False, stop=(ct == FT - 1),
                            )
                return pt

            def w3_mms(b, h):
                pt = psum.tile([P, FT, CH], F32, tag="mm")
                for j in range(CH // HALF):
                    js = slice(j * HALF, (j + 1) * HALF)
                    for ct in range(FT):
                        nc.tensor.matmul(
                            pt[0:1, 0, js], w3[b][:, ct:ct + 1], h[:, ct, js],
                            start=(ct == 0), stop=(ct == FT - 1),
                        )
                return pt

            def act(pt, tag):
                h = hbuf.tile([P, FT, CH], F32R, tag="h")
                nc.scalar.activation(h[:], pt[:], AF.Tanh)
                return h

            # software pipeline: two blocks in flight, stage-interleaved.
            for p0 in range(0, nblk, 2):
                blks = [b for b in (p0, p0 + 1) if b < nblk]
                ps = {b: l0_mms(b) for b in blks}
                h0 = {b: act(ps[b], "h0") for b in blks}
                ps = {b: hidden_mms(b, w1, b1, h0[b]) for b in blks}
                h1 = {b: act(ps[b], "h1") for b in blks}
                ps = {b: hidden_mms(b, w2, b2, h1[b]) for b in blks}
                h2 = {b: act(ps[b], "h2") for b in blks}
                ps = {b: w3_mms(b, h2[b]) for b in blks}
                for b in blks:
                    st = stage.tile([1, CH], F32, tag="out")
                    nc.vector.tensor_copy(st[:], ps[b][0:1, 0, :])
                    nc.sync.dma_start(outd[b], st[:])

    nc.compile()
    return nc


_PROGRAMS = {}
_LAST = {}


def _program(nblk=None):
    if nblk is None:
        nblk = _LAST.get("nblk", 11)
    if nblk not in _PROGRAMS:
        _PROGRAMS[nblk] = _build_program(nblk)
    return _PROGRAMS[nblk]


def _route(x, xmins, xmaxs):
    """Blocks of active points per subdomain + their window weights."""
    x64 = x.astype(np.float64)
    blocks = []  # (k, idx[int32] padded to CH, real_len, wvals[real_len])
    for k in range(xmins.shape[0]):
        lo = xmins[k].astype(np.float64) - TW
        hi = xmaxs[k].astype(np.float64) + TW
        mask = np.all((x64 > lo) & (x64 < hi), axis=1)
        idx = np.nonzero(mask)[0].astype(np.int64)
        if idx.size == 0:
            continue
        t_l = np.clip((x64[idx] - lo) / (2.0 * TW), 0.0, 1.0)
        t_r = np.clip((hi - x64[idx]) / (2.0 * TW), 0.0, 1.0)
        wv = np.prod(0.25 * (1.0 - np.cos(np.pi * t_l))
                     * (1.0 - np.cos(np.pi * t_r)), axis=1)
        for c0 in range(0, idx.size, CH):
            ci = idx[c0:c0 + CH]
            real = ci.size
            if real < CH:
                ci = np.concatenate([ci, np.full(CH - real, idx[0])])
            blocks.append((k, ci, real, wv[c0:c0 + real]))
    return blocks


def _prep_in_maps(x, W0, b0, W1, b1, W2, b2, W3, b3, xmins, xmaxs):
    f32 = np.float32
    x = np.asarray(x, f32)
    center = ((xmins + xmaxs) * 0.5).astype(f32)
    scale = np.maximum((xmaxs - xmins) * 0.5, 1e-9).astype(f32)

    blocks = _route(x, xmins, xmaxs)
    per_core = [[] for _ in range(NCORES)]
    for j, blk in enumerate(blocks):
        per_core[j % NCORES].append(blk)
    nblk = max(len(c) for c in per_core)
    for c in per_core:
        while len(c) < nblk:
            k, ci, _, _ = c[0]
            c.append((k, ci, 0, np.zeros(0)))  # dummy, output ignored

    in_maps, meta = [], []
    for core in range(NCORES):
        xas = np.zeros((3, nblk * CH), f32)
        w0s = np.zeros((35, nblk * P), f32)
        w1s = np.zeros((P, nblk * FT * FT, P), f32)
        b1s = np.zeros((33, nblk * P), f32)
        w2s = np.zeros((P, nblk * FT * FT, P), f32)
        b2s = np.zeros((33, nblk * P), f32)
        w3s = np.zeros((P, nblk * FT), f32)
        cmeta = []
        for b, (k, ci, real, wv) in enumerate(per_core[core]):
            xn = (x[ci] - center[k]) / scale[k]       # [CH, 2]
            xas[0:2, b * CH:(b + 1) * CH] = xn.T
            xas[2, b * CH:(b + 1) * CH] = 1.0
            w0eff = W0[k].astype(f32)   # xa already holds normalized coords
            b0eff = b0[k].astype(f32)
            for mt in range(FT):
                r0 = 0 if mt == 0 else 32
                w0s[r0:r0 + 2, b * P:(b + 1) * P] = w0eff[:, mt * P:(mt + 1) * P]
                w0s[r0 + 2, b * P:(b + 1) * P] = b0eff[mt * P:(mt + 1) * P]
                b1s[r0, b * P:(b + 1) * P] = b1[k][mt * P:(mt + 1) * P]
                b2s[r0, b * P:(b + 1) * P] = b2[k][mt * P:(mt + 1) * P]
                w3s[:, b * FT + mt] = W3[k][mt * P:(mt + 1) * P, 0]
                for ct in range(FT):
                    w1s[:, b * FT * FT + mt * FT + ct, :] = (
                        W1[k][ct * P:(ct + 1) * P, mt * P:(mt + 1) * P])
                    w2s[:, b * FT * FT + mt * FT + ct, :] = (
                        W2[k][ct * P:(ct + 1) * P, mt * P:(mt + 1) * P])
            cmeta.append((k, ci, real, wv))
        in_maps.append({
            "XA": xas, "W0S": w0s, "W1S": w1s, "B1S": b1s,
            "W2S": w2s, "B2S": b2s, "W3S": w3s,
        })
        meta.append(cmeta)

    _LAST.update(nblk=nblk, meta=meta, b3=np.asarray(b3, np.float64))
    return in_maps


def kernel(x, W0, b0, W1, b1, W2, b2, W3, b3, xmins, xmaxs):
    args = [np.asarray(a, np.float32) for a in
            (x, W0, b0, W1, b1, W2, b2, W3, b3, xmins, xmaxs)]
    in_maps = _prep_in_maps(*args)
    nc = _program(_LAST["nblk"])
    res = run_bass_kernel_spmd(nc, in_maps, list(range(NCORES)))

    n = x.shape[0]
    num = np.zeros(n, np.float64)
    den = np.zeros(n, np.float64)
    b3f = _LAST["b3"]
    for core in range(NCORES):
        out = np.asarray(res.results[core]["OUT"], np.float64)  # [nblk,1,CH]
        for b, (k, ci, real, wv) in enumerate(_LAST["meta"][core]):
            if real == 0:
                continue
            sub = out[b, 0, :real] + b3f[k, 0]
            np.add.at(num, ci[:real], wv * sub)
            np.add.at(den, ci[:real], wv)
    result = (num / (den + 1e-9)).astype(np.float32)
    return result.reshape(n, OUT_DIM)


# revision 5
# speedup vs baseline: 1.5141x; 1.1281x over previous
"""FBPINN (16 subdomain MLPs over [0,1]^2, cosine partition-of-unity windows)
as a Trainium2 Bass kernel with MoE-style routing across 8 NeuronCores.

Key idea: each subdomain's window is exactly zero outside its support box
(xmin-TW, xmax+TW), so its MLP only needs to run on the ~20-42% of points
inside that box. The host routes: it gathers each subdomain's active points
into 1024-point blocks (~84 blocks total vs 256 dense), load-balances the
blocks across the 8 cores, and does the final window-weighted scatter/
normalize. The device runs the heavy part: per block, a 3-layer tanh MLP
(256 wide) plus the W3 contraction.

Engine split per block: TensorE does layer matmuls (layer 0 in f32r with
the b0 bias folded in as a K=3 ones-row; hidden layers in bf16 with
1024-wide moving operands so each (mt,ct) tile is a single self-loading
matmul); VectorE adds the b1/b2 biases directly into PSUM; ScalarE then
applies tanh over both 128-feature tiles of a layer in one fused
2048-element instruction. Two blocks are pipelined stage-locked so PE and
ACT overlap; PSUM holds exactly two 4-bank accumulator tiles.
"""

import numpy as np
import ml_dtypes

import concourse.bacc as bacc
import concourse.mybir as mybir
import concourse.tile as tile
from concourse.bass_utils import run_bass_kernel_spmd

K, D, N, W, OUT_DIM = 16, 2, 16384, 256, 1
TW = 0.2
NCORES = 8
P = 128
CH = 1024          # points per block
HALF = 512         # f32r matmul moving-operand subchunk (one PSUM bank)
FT = W // P        # feature tiles per hidden layer (2)

F32 = mybir.dt.float32
F32R = mybir.dt.float32r
BF16 = mybir.dt.bfloat16
AF = mybir.ActivationFunctionType
ALU = mybir.AluOpType
BF16NP = ml_dtypes.bfloat16


def _build_program(nblk):
    nc = bacc.Bacc("TRN2", target_bir_lowering=False, debug=False,
                   num_devices=NCORES)

    xad = nc.dram_tensor("XA", [3, nblk * CH], F32R, kind="ExternalInput")
    w0d = nc.dram_tensor("W0S", [35, nblk * P], F32R, kind="ExternalInput")
    w1d = nc.dram_tensor("W1S", [P, nblk * FT * FT, P], BF16, kind="ExternalInput")
    b1d = nc.dram_tensor("B1S", [P, nblk * FT], F32, kind="ExternalInput")
    w2d = nc.dram_tensor("W2S", [P, nblk * FT * FT, P], BF16, kind="ExternalInput")
    b2d = nc.dram_tensor("B2S", [P, nblk * FT], F32, kind="ExternalInput")
    w3d = nc.dram_tensor("W3S", [P, nblk * FT], BF16, kind="ExternalInput")
    outd = nc.dram_tensor("OUT", [nblk, 1, CH], F32, kind="ExternalOutput")

    with tile.TileContext(nc) as tc:
        with (
            tc.tile_pool(name="xin", bufs=nblk) as xin,
            tc.tile_pool(name="wgt", bufs=nblk) as wgt,
            tc.tile_pool(name="hbuf", bufs=4) as hbuf,
            tc.tile_pool(name="stage", bufs=4) as stage,
            tc.tile_pool(name="psum", bufs=2, space="PSUM") as psum,
        ):
            # per-block inputs, all DMAs issued up front (block-major so
            # early blocks unblock quickly); big hidden weights on the
            # gpsimd queue, the rest on sync.
            xa, w0, w1, b1, w2, b2, w3 = [], [], [], [], [], [], []
            for b in range(nblk):
                xt = xin.tile([35, CH], F32R, tag="xa")
                nc.sync.dma_start(xt[0:3, :], xad[:, b * CH:(b + 1) * CH])
                nc.sync.dma_start(xt[32:35, :], xt[0:3, :])
                xa.append(xt)
                w0t = wgt.tile([35, P], F32R, tag="w0")
                nc.sync.dma_start(w0t[:], w0d[:, b * P:(b + 1) * P])
                w0.append(w0t)
                b1t = wgt.tile([P, FT], F32, tag="b1")
                nc.sync.dma_start(b1t[:], b1d[:, b * FT:(b + 1) * FT])
                b1.append(b1t)
                b2t = wgt.tile([P, FT], F32, tag="b2")
                nc.sync.dma_start(b2t[:], b2d[:, b * FT:(b + 1) * FT])
                b2.append(b2t)
                w3t = wgt.tile([P, FT], BF16, tag="w3")
                nc.sync.dma_start(w3t[:], w3d[:, b * FT:(b + 1) * FT])
                w3.append(w3t)
                w1t = wgt.tile([P, FT * FT, P], BF16, tag="w1")
                nc.gpsimd.dma_start(w1t[:], w1d[:, b * FT * FT:(b + 1) * FT * FT, :])
                w1.append(w1t)
                w2t = wgt.tile([P, FT * FT, P], BF16, tag="w2")
                nc.gpsimd.dma_start(w2t[:], w2d[:, b * FT * FT:(b + 1) * FT * FT, :])
                w2.append(w2t)

            def l0_mms(b):
                # layer 0: K=3 f32r (two normalized coords + ones row
                # carrying b0); mt pair packed into PE row groups 0 / 32.
                # Loops are weight-major: consecutive matmuls share the
                # stationary operand so the weight load is paid once.
                pt = psum.tile([P, FT, CH], F32, tag="mm")
                for mt, r0 in ((0, 0), (1, 32)):
                    for j in range(CH // HALF):
                        js = slice(j * HALF, (j + 1) * HALF)
                        nc.tensor.matmul(
                            pt[:, mt, js], w0[b][r0:r0 + 3, :], xa[b][r0:r0 + 3, js],
                            start=True, stop=True, tile_position=(r0, 0))
                return pt

            def hidden_mms(b, wl, bl, h):
                # bf16 weights, weight-major loops; VectorE then adds the
                # bias into PSUM so the tanh stays fused across both mts.
                pt = psum.tile([P, FT, CH], F32, tag="mm")
                for mt in range(FT):
                    for ct in range(FT):
                        for j in range(CH // HALF):
                            js = slice(j * HALF, (j + 1) * HALF)
                            nc.tensor.matmul(
                                pt[:, mt, js], wl[b][:, mt * FT + ct, :],
                                h[:, ct, js],
                                start=(ct == 0), stop=(ct == FT - 1),
                            )
                for mt in range(FT):
                    nc.vector.tensor_scalar_add(
                        pt[:, mt, :], pt[:, mt, :], bl[b][:, mt:mt + 1])
                return pt

            def w3_mms(b, h):
                pt = psum.tile([P, FT, CH], F32, tag="mm")
                for ct in range(FT):
                    for j in range(CH // HALF):
                        js = slice(j * HALF, (j + 1) * HALF)
                        nc.tensor.matmul(
                            pt[0:1, 0, js], w3[b][:, ct:ct + 1], h[:, ct, js],
                            start=(ct == 0), stop=(ct == FT - 1),
                        )
                return pt

            def act(pt):
                h = hbuf.tile([P, FT, CH], BF16, tag="h")
                nc.scalar.activation(h[:], pt[:], AF.Tanh)
                return h

            # software pipeline: two blocks in flight, stage-interleaved.
            for p0 in range(0, nblk, 2):
                blks = [b for b in (p0, p0 + 1) if b < nblk]
                ps = {b: l0_mms(b) for b in blks}
                h0 = {b: act(ps[b]) for b in blks}
                ps = {b: hidden_mms(b, w1, b1, h0[b]) for b in blks}
                h1 = {b: act(ps[b]) for b in blks}
                ps = {b: hidden_mms(b, w2, b2, h1[b]) for b in blks}
                h2 = {b: act(ps[b]) for b in blks}
                ps = {b: w3_mms(b, h2[b]) for b in blks}
                for b in blks:
                    st = stage.tile([1, CH], F32, tag="out")
                    nc.vector.tensor_copy(st[:], ps[b][0:1, 0, :])
                    nc.sync.dma_start(outd[b], st[:])

    nc.compile()
    return nc


_PROGRAMS = {}
_LAST = {}


def _program(nblk=None):
    if nblk is None:
        nblk = _LAST.get("nblk", 11)
    if nblk not in _PROGRAMS:
        _PROGRAMS[nblk] = _build_program(nblk)
    return _PROGRAMS[nblk]


def _route(x, xmins, xmaxs):
    """Blocks of active points per subdomain + their window weights."""
    x64 = x.astype(np.float64)
    blocks = []  # (k, idx[int] padded to CH, real_len, wvals[real_len])
    for k in range(xmins.shape[0]):
        lo = xmins[k].astype(np.float64) - TW
        hi = xmaxs[k].astype(np.float64) + TW
        mask = np.all((x64 > lo) & (x64 < hi), axis=1)
        idx = np.nonzero(mask)[0].astype(np.int64)
        if idx.size == 0:
            continue
        t_l = np.clip((x64[idx] - lo) / (2.0 * TW), 0.0, 1.0)
        t_r = np.clip((hi - x64[idx]) / (2.0 * TW), 0.0, 1.0)
        wv = np.prod(0.25 * (1.0 - np.cos(np.pi * t_l))
                     * (1.0 - np.cos(np.pi * t_r)), axis=1)
        for c0 in range(0, idx.size, CH):
            ci = idx[c0:c0 + CH]
            real = ci.size
            if real < CH:
                ci = np.concatenate([ci, np.full(CH - real, idx[0])])
            blocks.append((k, ci, real, wv[c0:c0 + real]))
    return blocks


def _prep_in_maps(x, W0, b0, W1, b1, W2, b2, W3, b3, xmins, xmaxs):
    f32 = np.float32
    x = np.asarray(x, f32)
    center = ((xmins + xmaxs) * 0.5).astype(f32)
    scale = np.maximum((xmaxs - xmins) * 0.5, 1e-9).astype(f32)

    blocks = _route(x, xmins, xmaxs)
    per_core = [[] for _ in range(NCORES)]
    for j, blk in enumerate(blocks):
        per_core[j % NCORES].append(blk)
    nblk = max(len(c) for c in per_core)
    for c in per_core:
        while len(c) < nblk:
            k, ci, _, _ = c[0]
            c.append((k, ci, 0, np.zeros(0)))  # dummy, output ignored

    in_maps, meta = [], []
    for core in range(NCORES):
        xas = np.zeros((3, nblk * CH), f32)
        w0s = np.zeros((35, nblk * P), f32)
        w1s = np.zeros((P, nblk * FT * FT, P), f32)
        b1s = np.zeros((P, nblk * FT), f32)
        w2s = np.zeros((P, nblk * FT * FT, P), f32)
        b2s = np.zeros((P, nblk * FT), f32)
        w3s = np.zeros((P, nblk * FT), f32)
        cmeta = []
        for b, (k, ci, real, wv) in enumerate(per_core[core]):
            xn = (x[ci] - center[k]) / scale[k]       # [CH, 2]
            xas[0:2, b * CH:(b + 1) * CH] = xn.T
            xas[2, b * CH:(b + 1) * CH] = 1.0
            for mt in range(FT):
                r0 = 0 if mt == 0 else 32
                w0s[r0:r0 + 2, b * P:(b + 1) * P] = W0[k][:, mt * P:(mt + 1) * P]
                w0s[r0 + 2, b * P:(b + 1) * P] = b0[k][mt * P:(mt + 1) * P]
                b1s[:, b * FT + mt] = b1[k][mt * P:(mt + 1) * P]
                b2s[:, b * FT + mt] = b2[k][mt * P:(mt + 1) * P]
                w3s[:, b * FT + mt] = W3[k][mt * P:(mt + 1) * P, 0]
                for ct in range(FT):
                    w1s[:, b * FT * FT + mt * FT + ct, :] = (
                        W1[k][ct * P:(ct + 1) * P, mt * P:(mt + 1) * P])
                    w2s[:, b * FT * FT + mt * FT + ct, :] = (
                        W2[k][ct * P:(ct + 1) * P, mt * P:(mt + 1) * P])
            cmeta.append((k, ci, real, wv))
        in_maps.append({
            "XA": xas, "W0S": w0s,
            "W1S": w1s.astype(BF16NP), "B1S": b1s,
            "W2S": w2s.astype(BF16NP), "B2S": b2s,
            "W3S": w3s.astype(BF16NP),
        })
        meta.append(cmeta)

    _LAST.update(nblk=nblk, meta=meta, b3=np.asarray(b3, np.float64))
    return in_maps


def kernel(x, W0, b0, W1, b1, W2, b2, W3, b3, xmins, xmaxs):
    args = [np.asarray(a, np.float32) for a in
            (x, W0, b0, W1, b1, W2, b2, W3, b3, xmins, xmaxs)]
    in_maps = _prep_in_maps(*args)
    nc = _program(_LAST["nblk"])
    res = run_bass_kernel_spmd(nc, in_maps, list(range(NCORES)))

    n = x.shape[0]
    num = np.zeros(n, np.float64)
    den = np.zeros(n, np.float64)
    b3f = _LAST["b3"]
    for core in range(NCORES):
        out = np.asarray(res.results[core]["OUT"], np.float64)  # [nblk,1,CH]
        for b, (k, ci, real, wv) in enumerate(_LAST["meta"][core]):
            if real == 0:
                continue
            sub = out[b, 0, :real] + b3f[k, 0]
            np.add.at(num, ci[:real], wv * sub)
            np.add.at(den, ci[:real], wv)
    result = (num / (den + 1e-9)).astype(np.float32)
    return result.reshape(n, OUT_DIM)


# revision 7
# speedup vs baseline: 1.9481x; 1.2866x over previous
"""FBPINN (16 subdomain MLPs over [0,1]^2, cosine partition-of-unity windows)
as a Trainium2 Bass kernel with MoE-style routing across 8 NeuronCores.

Key idea: each subdomain's window is exactly zero outside its support box
(xmin-TW, xmax+TW), so its MLP only needs to run on the ~20-42% of points
inside that box. The host routes: it gathers each subdomain's active points
into 1024-point blocks (~84 blocks total vs 256 dense), load-balances the
blocks across the 8 cores, and does the final window-weighted scatter/
normalize. The device runs the heavy part: per block, a 3-layer tanh MLP
(256 wide) plus the W3 contraction.

Engine split per block: TensorE does all matmuls — layer 0 in f32r with
the b0 bias folded in as a K=3 ones-row, hidden-layer b1/b2 as K=1 bf16
bias matmuls (PE row groups 0/32) that the bf16 contraction tiles then
accumulate onto; ScalarE applies tanh over both 128-feature tiles of a
layer in one fused 2048-element instruction. Two blocks are pipelined
stage-locked so PE and ACT overlap and the PE clock stays un-throttled;
PSUM holds exactly two 4-bank accumulator tiles.
"""

import numpy as np
import ml_dtypes

import concourse.bacc as bacc
import concourse.mybir as mybir
import concourse.tile as tile
from concourse.bass_utils import run_bass_kernel_spmd

K, D, N, W, OUT_DIM = 16, 2, 16384, 256, 1
TW = 0.2
NCORES = 8
P = 128
CH = 1024          # points per block
HALF = 512         # f32r matmul moving-operand subchunk (one PSUM bank)
FT = W // P        # feature tiles per hidden layer (2)

F32 = mybir.dt.float32
F32R = mybir.dt.float32r
BF16 = mybir.dt.bfloat16
AF = mybir.ActivationFunctionType
ALU = mybir.AluOpType
BF16NP = ml_dtypes.bfloat16


def _build_program(nblk):
    nc = bacc.Bacc("TRN2", target_bir_lowering=False, debug=False,
                   num_devices=NCORES)

    xad = nc.dram_tensor("XA", [3, nblk * CH], F32R, kind="ExternalInput")
    w0d = nc.dram_tensor("W0S", [35, nblk * P], F32R, kind="ExternalInput")
    w1d = nc.dram_tensor("W1S", [P, nblk * FT * FT, P], BF16, kind="ExternalInput")
    b1d = nc.dram_tensor("B1S", [33, nblk * P], BF16, kind="ExternalInput")
    w2d = nc.dram_tensor("W2S", [P, nblk * FT * FT, P], BF16, kind="ExternalInput")
    b2d = nc.dram_tensor("B2S", [33, nblk * P], BF16, kind="ExternalInput")
    w3d = nc.dram_tensor("W3S", [P, nblk * FT], BF16, kind="ExternalInput")
    outd = nc.dram_tensor("OUT", [nblk, 1, CH], F32, kind="ExternalOutput")

    with tile.TileContext(nc) as tc:
        with (
            tc.tile_pool(name="xin", bufs=nblk) as xin,
            tc.tile_pool(name="wgt", bufs=nblk) as wgt,
            tc.tile_pool(name="hbuf", bufs=4) as hbuf,
            tc.tile_pool(name="stage", bufs=4) as stage,
            tc.tile_pool(name="psum", bufs=2, space="PSUM") as psum,
        ):
            ones = xin.tile([33, HALF], BF16, bufs=1, tag="ones")
            nc.vector.memset(ones[:], 1.0)

            # per-block inputs, all DMAs issued up front (block-major so
            # early blocks unblock quickly); big hidden weights on the
            # gpsimd queue, the rest on sync.
            xa, w0, w1, b1, w2, b2, w3 = [], [], [], [], [], [], []
            for b in range(nblk):
                xt = xin.tile([35, CH], F32R, tag="xa")
                nc.sync.dma_start(xt[0:3, :], xad[:, b * CH:(b + 1) * CH])
                nc.sync.dma_start(xt[32:35, :], xt[0:3, :])
                xa.append(xt)
                w0t = wgt.tile([35, P], F32R, tag="w0")
                nc.sync.dma_start(w0t[:], w0d[:, b * P:(b + 1) * P])
                w0.append(w0t)
                b1t = wgt.tile([33, P], BF16, tag="b1")
                nc.sync.dma_start(b1t[:], b1d[:, b * P:(b + 1) * P])
                b1.append(b1t)
                b2t = wgt.tile([33, P], BF16, tag="b2")
                nc.sync.dma_start(b2t[:], b2d[:, b * P:(b + 1) * P])
                b2.append(b2t)
                w3t = wgt.tile([P, FT], BF16, tag="w3")
                nc.sync.dma_start(w3t[:], w3d[:, b * FT:(b + 1) * FT])
                w3.append(w3t)
                w1t = wgt.tile([P, FT * FT, P], BF16, tag="w1")
                nc.gpsimd.dma_start(w1t[:], w1d[:, b * FT * FT:(b + 1) * FT * FT, :])
                w1.append(w1t)
                w2t = wgt.tile([P, FT * FT, P], BF16, tag="w2")
                nc.gpsimd.dma_start(w2t[:], w2d[:, b * FT * FT:(b + 1) * FT * FT, :])
                w2.append(w2t)

            def l0_mms(b):
                # layer 0: K=3 f32r (two normalized coords + ones row
                # carrying b0); mt pair packed into PE row groups 0 / 32.
                # Loops are weight-major: consecutive matmuls share the
                # stationary operand so the weight load is paid once.
                pt = psum.tile([P, FT, CH], F32, tag="mm")
                for mt, r0 in ((0, 0), (1, 32)):
                    for j in range(CH // HALF):
                        js = slice(j * HALF, (j + 1) * HALF)
                        nc.tensor.matmul(
                            pt[:, mt, js], w0[b][r0:r0 + 3, :], xa[b][r0:r0 + 3, js],
                            start=True, stop=True, tile_position=(r0, 0))
                return pt

            def hidden_mms(b, wl, bl, h):
                # bias rows first via K=1 matmuls (PE row groups 0/32 so
                # the mt pair runs concurrently), then the bf16 K=128
                # contraction tiles accumulate on top; tanh stays fused.
                pt = psum.tile([P, FT, CH], F32, tag="mm")
                for j in range(CH // HALF):
                    js = slice(j * HALF, (j + 1) * HALF)
                    for mt, r0 in ((0, 0), (1, 32)):
                        nc.tensor.matmul(
                            pt[:, mt, js], bl[b][r0:r0 + 1, :], ones[r0:r0 + 1, :],
                            start=True, stop=False, tile_position=(r0, 0))
                for mt in range(FT):
                    for ct in range(FT):
                        for j in range(CH // HALF):
                            js = slice(j * HALF, (j + 1) * HALF)
                            nc.tensor.matmul(
                                pt[:, mt, js], wl[b][:, mt * FT + ct, :],
                                h[:, ct, js],
                                start=False, stop=(ct == FT - 1),
                            )
                return pt

            def w3_mms(b, h):
                pt = psum.tile([P, FT, CH], F32, tag="mm")
                for ct in range(FT):
                    for j in range(CH // HALF):
                        js = slice(j * HALF, (j + 1) * HALF)
                        nc.tensor.matmul(
                            pt[0:1, 0, js], w3[b][:, ct:ct + 1], h[:, ct, js],
                            start=(ct == 0), stop=(ct == FT - 1),
                        )
                return pt

            def act(pt):
                h = hbuf.tile([P, FT, CH], BF16, tag="h")
                nc.scalar.activation(h[:], pt[:], AF.Tanh)
                return h

            # software pipeline: two blocks in flight, stage-interleaved.
            for p0 in range(0, nblk, 2):
                blks = [b for b in (p0, p0 + 1) if b < nblk]
                ps = {b: l0_mms(b) for b in blks}
                h0 = {b: act(ps[b]) for b in blks}
                ps = {b: hidden_mms(b, w1, b1, h0[b]) for b in blks}
                h1 = {b: act(ps[b]) for b in blks}
                ps = {b: hidden_mms(b, w2, b2, h1[b]) for b in blks}
                h2 = {b: act(ps[b]) for b in blks}
                ps = {b: w3_mms(b, h2[b]) for b in blks}
                for b in blks:
                    st = stage.tile([1, CH], F32, tag="out")
                    nc.vector.tensor_copy(st[:], ps[b][0:1, 0, :])
                    nc.sync.dma_start(outd[b], st[:])

    nc.compile()
    return nc


_PROGRAMS = {}
_LAST = {}


def _program(nblk=None):
    if nblk is None:
        nblk = _LAST.get("nblk", 11)
    if nblk not in _PROGRAMS:
        _PROGRAMS[nblk] = _build_program(nblk)
    return _PROGRAMS[nblk]


def _route(x, xmins, xmaxs):
    """Blocks of active points per subdomain + their window weights."""
    x64 = x.astype(np.float64)
    blocks = []  # (k, idx[int] padded to CH, real_len, wvals[real_len])
    for k in range(xmins.shape[0]):
        lo = xmins[k].astype(np.float64) - TW
        hi = xmaxs[k].astype(np.float64) + TW
        mask = np.all((x64 > lo) & (x64 < hi), axis=1)
        idx = np.nonzero(mask)[0].astype(np.int64)
        if idx.size == 0:
            continue
        t_l = np.clip((x64[idx] - lo) / (2.0 * TW), 0.0, 1.0)
        t_r = np.clip((hi - x64[idx]) / (2.0 * TW), 0.0, 1.0)
        wv = np.prod(0.25 * (1.0 - np.cos(np.pi * t_l))
                     * (1.0 - np.cos(np.pi * t_r)), axis=1)
        for c0 in range(0, idx.size, CH):
            ci = idx[c0:c0 + CH]
            real = ci.size
            if real < CH:
                ci = np.concatenate([ci, np.full(CH - real, idx[0])])
            blocks.append((k, ci, real, wv[c0:c0 + real]))
    return blocks


def _prep_in_maps(x, W0, b0, W1, b1, W2, b2, W3, b3, xmins, xmaxs):
    f32 = np.float32
    x = np.asarray(x, f32)
    center = ((xmins + xmaxs) * 0.5).astype(f32)
    scale = np.maximum((xmaxs - xmins) * 0.5, 1e-9).astype(f32)

    blocks = _route(x, xmins, xmaxs)
    per_core = [[] for _ in range(NCORES)]
    for j, blk in enumerate(blocks):
        per_core[j % NCORES].append(blk)
    nblk = max(len(c) for c in per_core)
    for c in per_core:
        while len(c) < nblk:
            k, ci, _, _ = c[0]
            c.append((k, ci, 0, np.zeros(0)))  # dummy, output ignored

    in_maps, meta = [], []
    for core in range(NCORES):
        xas = np.zeros((3, nblk * CH), f32)
        w0s = np.zeros((35, nblk * P), f32)
        w1s = np.zeros((P, nblk * FT * FT, P), f32)
        b1s = np.zeros((33, nblk * P), f32)
        w2s = np.zeros((P, nblk * FT * FT, P), f32)
        b2s = np.zeros((33, nblk * P), f32)
        w3s = np.zeros((P, nblk * FT), f32)
        cmeta = []
        for b, (k, ci, real, wv) in enumerate(per_core[core]):
            xn = (x[ci] - center[k]) / scale[k]       # [CH, 2]
            xas[0:2, b * CH:(b + 1) * CH] = xn.T
            xas[2, b * CH:(b + 1) * CH] = 1.0
            for mt in range(FT):
                r0 = 0 if mt == 0 else 32
                w0s[r0:r0 + 2, b * P:(b + 1) * P] = W0[k][:, mt * P:(mt + 1) * P]
                w0s[r0 + 2, b * P:(b + 1) * P] = b0[k][mt * P:(mt + 1) * P]
                b1s[r0, b * P:(b + 1) * P] = b1[k][mt * P:(mt + 1) * P]
                b2s[r0, b * P:(b + 1) * P] = b2[k][mt * P:(mt + 1) * P]
                w3s[:, b * FT + mt] = W3[k][mt * P:(mt + 1) * P, 0]
                for ct in range(FT):
                    w1s[:, b * FT * FT + mt * FT + ct, :] = (
                        W1[k][ct * P:(ct + 1) * P, mt * P:(mt + 1) * P])
                    w2s[:, b * FT * FT + mt * FT + ct, :] = (
                        W2[k][ct * P:(ct + 1) * P, mt * P:(mt + 1) * P])
            cmeta.append((k, ci, real, wv))
        in_maps.append({
            "XA": xas, "W0S": w0s,
            "W1S": w1s.astype(BF16NP), "B1S": b1s.astype(BF16NP),
            "W2S": w2s.astype(BF16NP), "B2S": b2s.astype(BF16NP),
            "W3S": w3s.astype(BF16NP),
        })
        meta.append(cmeta)

    _LAST.update(nblk=nblk, meta=meta, b3=np.asarray(b3, np.float64))
    return in_maps


def kernel(x, W0, b0, W1, b1, W2, b2, W3, b3, xmins, xmaxs):
    args = [np.asarray(a, np.float32) for a in
            (x, W0, b0, W1, b1, W2, b2, W3, b3, xmins, xmaxs)]
    in_maps = _prep_in_maps(*args)
    nc = _program(_LAST["nblk"])
    res = run_bass_kernel_spmd(nc, in_maps, list(range(NCORES)))

    n = x.shape[0]
    num = np.zeros(n, np.float64)
    den = np.zeros(n, np.float64)
    b3f = _LAST["b3"]
    for core in range(NCORES):
        out = np.asarray(res.results[core]["OUT"], np.float64)  # [nblk,1,CH]
        for b, (k, ci, real, wv) in enumerate(_LAST["meta"][core]):
            if real == 0:
                continue
            sub = out[b, 0, :real] + b3f[k, 0]
            np.add.at(num, ci[:real], wv * sub)
            np.add.at(den, ci[:real], wv)
    result = (num / (den + 1e-9)).astype(np.float32)
    return result.reshape(n, OUT_DIM)
